# revision 36
# baseline (speedup 1.0000x reference)
"""DigitCaps dynamic-routing kernel for Trainium2, 8 NeuronCores (SPMD).

Problem:  in_caps [64, 2048, 16] f32, W [2048, 32, 32, 16] f32
          u_hat[b,r,j,o] = sum_i W[r,j,o,i] * in_caps[b,r,i]
          3 routing iterations:
            c = softmax_j(b_ij);  s[b,j,o] = sum_r c[r,j] u_hat[b,r,j,o]
            v = squash_o(s);      b_ij += (1/BS) sum_{b,o} u_hat v
          returns v[..., None]  -> [64, 32, 32, 1]

Strategy (per core, routes sharded 256/core; K = (r,i) = 4096 rows):
  * W resident in SBUF as fp16 Wt[(r,i),(j,o)], p-major, 8+8 slices on
    the two HWDGE queues; pass1-it0 streams behind the load.  u_hat is
    never materialized.
  * pass1: the K contraction is column-tiled - even chunks accumulate
    into PSUM partitions 0-63, odd chunks into 64-127, concurrently
    (~2x PE).  The shared banks mean no MM may use start=True, so the
    s tile is DVE-zeroed first and every MM accumulates via the
    per-element has_written bits.  Both halves go out in one [128,1024]
    fp16 AllReduce; they are summed after the readback (both halves
    land on partitions 0-63).  One AR per iteration; it0's uniform-c
    1/J is folded into the squash factor.
  * squash: processed per j-half straight off the readback DMAs (the
    h=0 G matmuls start on the first half); square on DVE in bf16
    (fp32 range + 2x mode; fp16 would overflow on it0's raw s).  A
    dense burst of junk matmuls keyed on the first s2 half runs during
    the squash so the PE HAM clock is high when the G phase starts.
  * pass2 windows are emitted stage-blocked in 2-batch superblocks
    (G matmuls | W.G mult | o-tree to 4 | 4 selector matmuls -> b_ij |
    softmax | wc), with pass1(it+1) chunk pairs interleaved so wc tiles
    are consumed close to creation.  wc = Wt (.) c runs on DVE in 2x
    packed-fp16 mode via a pair-duplicated c2 layout (the broadcast
    over o keeps a last AP dim of [stride 1, count 2]); alternate
    batches are split 2-chunks-gpsimd / 2-chunks-DVE.
  * ACT only ever runs Copy/Sqrt/Exp (2 table slots; dummy sqrt ops
    re-pin the Sqrt table off the squash critical path each iteration).
"""

import numpy as np
import ml_dtypes

import concourse.bacc as bacc
import concourse.mybir as mybir
import concourse.tile as tile
from concourse.bass_utils import run_bass_kernel_spmd

BS, R, J, I, O = 64, 2048, 32, 16, 32
NUM_IT = 3
N_CORES = 8
R_LOC = R // N_CORES            # 256 routes per core
K_LOC = R_LOC * I               # 4096 contraction rows per core
NCHUNK = K_LOC // 128           # 32 chunks (8 routes x 16 i each)
JO = J * O                      # 1024
F32 = mybir.dt.float32
BF16 = mybir.dt.bfloat16
FP16 = mybir.dt.float16
AX = mybir.AxisListType
ALU = mybir.AluOpType
ACTF = mybir.ActivationFunctionType

# per-batch engine for wc = W*c: 'd' DVE (c2 pair trick, 2x), 'g' gpsimd
WC_ENG = ["d", "m", "d", "m", "d", "m", "d", "m"]


def _build_nc():
    nc = bacc.Bacc(trn_type="TRN2", target_bir_lowering=False, debug=False,
                   num_devices=N_CORES)
    wt = nc.dram_tensor("wt", [128, NCHUNK * JO], FP16, kind="ExternalInput")
    ut = nc.dram_tensor("ut", [128, NCHUNK * BS], FP16, kind="ExternalInput")
    un = nc.dram_tensor("un", [BS, K_LOC], FP16, kind="ExternalInput")
    sel = nc.dram_tensor("sel", [128, 128], FP16, kind="ExternalInput")
    vout = nc.dram_tensor("vout", [BS, JO], F32, kind="ExternalOutput")
    cc_in = [nc.dram_tensor(f"cc_in{i}", [128, JO], FP16)
             for i in range(NUM_IT)]
    cc_out = [nc.dram_tensor(f"cc_out{i}", [128, JO], FP16,
                             addr_space="Shared") for i in range(NUM_IT)]
    rg = [list(range(N_CORES))]

    with tile.TileContext(nc) as tc:
        with (
            tc.tile_pool(name="big", bufs=1) as big,
            tc.tile_pool(name="wc", bufs=4) as wcp,
            tc.tile_pool(name="gsb", bufs=3) as gsbp,
            tc.tile_pool(name="tmp", bufs=2) as tmpp,
            tc.tile_pool(name="tb", bufs=4) as tbp,
            tc.tile_pool(name="small", bufs=1) as small,
            tc.tile_pool(name="sq2", bufs=2) as sq2p,
            tc.tile_pool(name="sps", bufs=1, space="PSUM") as spsp,
            tc.tile_pool(name="gps", bufs=2, space="PSUM") as gpsp,
            tc.tile_pool(name="bpsum", bufs=1, space="PSUM") as bpsum,
        ):
            # ---- resident tensors ----
            w_sb = big.tile([128, NCHUNK, JO], FP16)      # 64KB/part
            ut_sb = big.tile([128, NCHUNK, BS], FP16)
            un_sb = big.tile([BS, K_LOC], FP16)
            sel_sb = big.tile([128, 128], FP16)            # selector (1/64)
            e_rep = big.tile([128, NCHUNK, J], FP16)      # exp(b) scratch
            c_rep = big.tile([128, NCHUNK, J], FP16)      # c_ij replicated
            c2 = big.tile([128, NCHUNK * J, 2], FP16)     # c pair-duplicated
            b_acc = bpsum.tile([128, NCHUNK, J], F32)     # persistent b_ij
            dumm = big.tile([64, 2], F32)                  # act-table pin

            # ---- input loads (sync + scalar HWDGE queues) ----
            wt_v = wt.ap().rearrange("p (c f) -> p c f", f=JO)
            ut_v = ut.ap().rearrange("p (c f) -> p c f", f=BS)
            nc.sync.dma_start(out=ut_sb, in_=ut_v)
            nc.scalar.dma_start(out=sel_sb, in_=sel.ap())
            _dengs = [nc.sync, nc.scalar]
            for sl in range(16):
                _dengs[sl % 2].dma_start(
                    out=w_sb[:, 2 * sl:2 * sl + 2, :],
                    in_=wt_v[:, 2 * sl:2 * sl + 2, :])
            # un is only needed by pass2 (after AR0) -> load last.
            nc.scalar.dma_start(out=un_sb, in_=un.ap())
            # Pin ACT table slots (Sqrt / Exp; Copy lives in both sets).
            nc.vector.memset(dumm, 1.0)
            nc.scalar.sqrt(dumm[:, 0:1], dumm[:, 1:2])
            nc.scalar.activation(dumm[:, 0:1], dumm[:, 1:2], ACTF.Exp)

            def emit_p1_pair(k, s_ps, rhs_a=None, rhs_b=None):
                """Chunks (2k, 2k+1) -> s halves on PSUM partition halves
                (column-tiled, concurrent)."""
                ca, cb = 2 * k, 2 * k + 1
                first, last = (k == 0), (k == 15)
                if rhs_a is None:
                    rhs_a = w_sb[:, ca, :]
                    rhs_b = w_sb[:, cb, :]
                # The two column-tile halves interleave in the same PSUM
                # banks, so no MM may use start=True (it would clear the
                # other half's accumulation bits).  The tile is DVE-zeroed
                # before the first MM; every MM then accumulates (elements
                # with has_written unset are simply overwritten = added to
                # the memset zeros either way).
                for h in range(2):
                    hs = slice(h * 512, (h + 1) * 512)
                    nc.tensor.matmul(
                        out=s_ps[0:BS, hs], lhsT=ut_sb[:, ca, :],
                        rhs=rhs_a[:, hs], start=False, stop=False,
                        skip_group_check=True)
                    nc.tensor.matmul(
                        out=s_ps[BS:128, hs], lhsT=ut_sb[:, cb, :],
                        rhs=rhs_b[:, hs], start=False, stop=(last and h == 1),
                        skip_group_check=True)

            def emit_s_evac_ar(it, s_ps):
                """Evacuate both halves into one [128,1024] fp16 AR; the
                halves are summed after the AR readback."""
                s_a = small.tile([128, JO], FP16, tag="s_evac")
                nc.scalar.copy(s_a, s_ps)
                nc.scalar.dma_start(out=cc_in[it].ap(), in_=s_a)
                nc.gpsimd.collective_compute(
                    "AllReduce", ALU.add, replica_groups=rg,
                    ins=[cc_in[it].ap()], outs=[cc_out[it].ap()])
                # re-pin the Sqrt table off the squash critical path
                nc.scalar.sqrt(dumm[:, 0:1], dumm[:, 1:2])

            def emit_squash(it, final=False):
                """cc_out -> combine halves -> squash, processed per
                j-half so downstream h=0 work starts ~4us earlier; v fp16
                (f32 final).  s was AllReduced RAW; it0's uniform-c 1/J is
                folded into the squash factor."""
                s2 = sq2p.tile([BS, J, O], FP16, tag="s2")
                s2f = s2.rearrange("p j o -> p (j o)")
                if final:
                    vt = small.tile([BS, J, O], F32, tag="vf32")
                else:
                    vt = small.tile([BS, J, O], FP16, tag=f"vr{it % 2}")
                for q in range(2):
                    js = slice(q * 16, q * 16 + 16)
                    cs = slice(q * 512, (q + 1) * 512)
                    s2a = sq2p.tile([BS, 512], FP16, tag=f"s2a{q}")
                    s2b = sq2p.tile([BS, 512], FP16, tag=f"s2b{q}")
                    nc.sync.dma_start(out=s2a, in_=cc_out[it].ap()[0:BS, cs])
                    nc.scalar.dma_start(out=s2b,
                                        in_=cc_out[it].ap()[BS:128, cs])
                    nc.vector.tensor_tensor(out=s2f[:, cs], in0=s2a,
                                            in1=s2b, op=ALU.add)
                    if q == 0 and not final:
                        # HAM promoter: dense junk matmuls keyed on the
                        # first s2 half run during the rest of the squash
                        # so the PE clock is high when the G phase starts.
                        pro = gpsp.tile([128, JO], F32, tag="gps")
                        for w in range(12):
                            nc.tensor.matmul(
                                out=pro[0:BS, 0:512], lhsT=un_sb[:, 0:BS],
                                rhs=s2f[:, 0:512], start=True, stop=True)
                    # ss must not be fp16 (it0's raw s^2 overflows); bf16
                    # keeps fp32 range and the DVE 2x mode.
                    ss = small.tile([BS, 16, O], BF16, tag=f"ss{q}")
                    nc.vector.tensor_tensor(out=ss, in0=s2[:, js, :],
                                            in1=s2[:, js, :], op=ALU.mult)
                    sq = small.tile([BS, 16], F32, tag=f"sq{q}")
                    nc.vector.tensor_reduce(out=sq, in_=ss, axis=AX.X,
                                            op=ALU.add)
                    if it == 0:
                        sq1 = small.tile([BS, 16], F32, tag=f"sqs{q}")
                        nc.vector.tensor_scalar_mul(sq1, sq, 1.0 / (J * J))
                        sq = sq1
                    rt = small.tile([BS, 16], F32, tag=f"rt{q}")
                    nc.scalar.sqrt(rt, sq)
                    op1 = small.tile([BS, 16], F32, tag=f"op{q}")
                    nc.vector.tensor_scalar_add(op1, sq, 1.0)
                    rden = small.tile([BS, 16], F32, tag=f"rd{q}")
                    nc.vector.reciprocal(rden, op1)
                    fac = small.tile([BS, 16], F32, tag=f"fa{q}")
                    nc.vector.tensor_tensor(out=fac, in0=rt, in1=rden,
                                            op=ALU.mult)
                    if it == 0:
                        fac1 = small.tile([BS, 16], F32, tag=f"fb{q}")
                        nc.vector.tensor_scalar_mul(fac1, fac, 1.0 / J)
                        fac = fac1
                    fac_b = fac.unsqueeze(2).broadcast_to([BS, 16, O])
                    nc.vector.tensor_tensor(out=vt[:, js, :],
                                            in0=s2[:, js, :], in1=fac_b,
                                            op=ALU.mult)
                    if final:
                        deng = nc.sync if q == 0 else nc.scalar
                        deng.dma_start(
                            out=vout.ap()[:, cs],
                            in_=vt.rearrange("p j o -> p (j o)")[:, cs])
                return vt.rearrange("p j o -> p (j o)")

            def emit_g_batch(b, v_r, g_tiles):
                """G = un.T @ v for the 4 chunks of batch b (PE) + ACT
                evacuation to fp16 SBUF."""
                c0 = 4 * b
                g_sb = gsbp.tile([128, 4, JO], FP16)
                for ci in range(4):
                    g_ps = gpsp.tile([128, JO], F32, tag="gps")
                    for h in range(2):
                        for m in range(2):
                            nc.tensor.matmul(
                                out=g_ps[m * 64:(m + 1) * 64,
                                         h * 512:(h + 1) * 512],
                                lhsT=un_sb[:, (c0 + ci) * 128 + m * 64:
                                           (c0 + ci) * 128 + m * 64 + 64],
                                rhs=v_r[:, h * 512:(h + 1) * 512],
                                start=True, stop=True)
                    nc.scalar.copy(g_sb[:, ci, :], g_ps)
                g_tiles[b] = g_sb

            def emit_mult_tree(b, g_tiles, tb_tiles):
                """tmp = W (.) G, fp16 tree to o=4 (DVE)."""
                c0 = 4 * b
                tmp = tmpp.tile([128, 4, JO], FP16)
                if b == 0:
                    # split the first batch's mult so DVE starts after only
                    # two G evacuations (shorter window pipeline fill)
                    for hf in range(2):
                        nc.vector.tensor_tensor(
                            out=tmp[:, 2 * hf:2 * hf + 2, :],
                            in0=w_sb[:, c0 + 2 * hf:c0 + 2 * hf + 2, :],
                            in1=g_tiles[b][:, 2 * hf:2 * hf + 2, :],
                            op=ALU.mult)
                else:
                    nc.vector.tensor_tensor(out=tmp,
                                            in0=w_sb[:, c0:c0 + 4, :],
                                            in1=g_tiles[b], op=ALU.mult)
                t0 = tmp.rearrange("p c (j o) -> p (c j) o", o=O)
                ta = tmpp.tile([128, 128, 16], FP16, tag="ta")
                nc.vector.tensor_tensor(out=ta, in0=t0[:, :, 0:16],
                                        in1=t0[:, :, 16:32], op=ALU.add)
                tb = tmpp.tile([128, 128, 8], FP16, tag="tb")
                nc.vector.tensor_tensor(out=tb, in0=ta[:, :, 0:8],
                                        in1=ta[:, :, 8:16], op=ALU.add)
                tcq = tbp.tile([128, 128, 4], FP16)
                nc.vector.tensor_tensor(out=tcq, in0=tb[:, :, 0:4],
                                        in1=tb[:, :, 4:8], op=ALU.add)
                tb_tiles[b] = tcq

            def emit_sel(it, b, tb_tiles):
                c0 = 4 * b
                tcq = tb_tiles[b]
                for oo in range(4):
                    nc.tensor.matmul(
                        out=b_acc[:, c0:c0 + 4, :], lhsT=sel_sb,
                        rhs=tcq[:, :, oo],
                        start=(it == 0 and c0 % 16 == 0 and oo == 0),
                        stop=(it == NUM_IT - 2 and c0 % 16 == 12
                              and oo == 3),
                        skip_group_check=True)

            def emit_softmax(b):
                c0 = 4 * b
                nc.scalar.activation(e_rep[:, c0:c0 + 4, :],
                                     b_acc[:, c0:c0 + 4, :], ACTF.Exp)
                esum = tmpp.tile([128, 4], F32, tag="esum")
                nc.vector.tensor_reduce(
                    out=esum, in_=e_rep[:, c0:c0 + 4, :],
                    axis=AX.X, op=ALU.add)
                erec = tmpp.tile([128, 4], F32, tag="erec")
                nc.vector.reciprocal(erec, esum)
                for cc in range(4):
                    nc.scalar.mul(c_rep[:, c0 + cc, :],
                                  e_rep[:, c0 + cc, :],
                                  erec[:, cc:cc + 1])

            def emit_c2_wc(b, wc_tiles):
                c0 = 4 * b
                wc_t = wcp.tile([128, 4, JO], FP16)
                nd = 4 if WC_ENG[b] == "d" else 2   # chunks on DVE (rest gp)
                if nd < 4:
                    # leading chunks on gpsimd, broadcast-direct
                    nc.gpsimd.tensor_tensor(
                        out=wc_t[:, 0:4 - nd, :].rearrange(
                            "p c (j o) -> p c j o", o=O),
                        in0=w_sb[:, c0:c0 + 4 - nd, :].rearrange(
                            "p c (j o) -> p c j o", o=O),
                        in1=c_rep[:, c0:c0 + 4 - nd, :].unsqueeze(3)
                            .broadcast_to([128, 4 - nd, J, O]),
                        op=ALU.mult)
                cd = c0 + 4 - nd                     # first DVE chunk
                cs = cd * J
                nc.scalar.copy(
                    c2[:, cs:cs + nd * J, :],
                    c_rep[:, cd:cd + nd, :]
                        .rearrange("p c j -> p (c j)").unsqueeze(2)
                        .broadcast_to([128, nd * J, 2]))
                nc.vector.tensor_tensor(
                    out=wc_t[:, 4 - nd:4, :].rearrange(
                        "p c (j oh ol) -> p (c j) oh ol", oh=16, ol=2),
                    in0=w_sb[:, cd:cd + nd, :].rearrange(
                        "p c (j oh ol) -> p (c j) oh ol", oh=16, ol=2),
                    in1=c2[:, cs:cs + nd * J, :].unsqueeze(2)
                        .broadcast_to([128, nd * J, 16, 2]),
                    op=ALU.mult)
                wc_tiles[b] = wc_t

            def emit_p1_batch(b, s_ps, wc_tiles):
                wc_t = wc_tiles[b]
                emit_p1_pair(2 * b, s_ps,
                             rhs_a=wc_t[:, 0, :], rhs_b=wc_t[:, 1, :])
                emit_p1_pair(2 * b + 1, s_ps,
                             rhs_a=wc_t[:, 2, :], rhs_b=wc_t[:, 3, :])

            def emit_window(it, v_r):
                """pass2(it) fused with pass1(it+1), emitted stage-blocked
                in 2-batch superblocks; returns new s psum."""
                s_ps = spsp.tile([128, JO], F32, tag="sps")
                nc.vector.memset(s_ps, 0.0)
                g_tiles, tb_tiles, wc_tiles = {}, {}, {}

                def stage(bs_g, bs_post, bs_p1):
                    for b in bs_g:
                        emit_g_batch(b, v_r, g_tiles)
                    for b in bs_post:
                        emit_mult_tree(b, g_tiles, tb_tiles)
                    for b in bs_post:
                        emit_sel(it, b, tb_tiles)
                    for b in bs_post:
                        emit_softmax(b)
                    for b in bs_post:
                        emit_c2_wc(b, wc_tiles)
                    for b in bs_p1:
                        emit_p1_batch(b, s_ps, wc_tiles)

                sbs = [(0, 1), (2, 3), (4, 5), (6, 7)]
                for s in range(6):
                    stage(sbs[s] if s < 4 else (),
                          sbs[s - 1] if 1 <= s < 5 else (),
                          sbs[s - 2] if s >= 2 else ())
                return s_ps

            # ---- iteration 0: pass1 on raw W, streaming behind the load
            s_ps = spsp.tile([128, JO], F32, tag="sps")
            nc.vector.memset(s_ps, 0.0)
            for k in range(16):
                emit_p1_pair(k, s_ps)
            emit_s_evac_ar(0, s_ps)

            for it in range(NUM_IT - 1):
                v_r = emit_squash(it)
                s_ps = emit_window(it, v_r)
                emit_s_evac_ar(it + 1, s_ps)

            # final squash (writes vout per half internally).
            emit_squash(NUM_IT - 1, final=True)
    nc.finalize()
    return nc


_NC_CACHE = {}
TRACE = False
TRACE_CORES = None


def _get_nc():
    if "nc" not in _NC_CACHE:
        _NC_CACHE["nc"] = _build_nc()
    return _NC_CACHE["nc"]


def _make_sel():
    sel = np.zeros((128, 128), np.float32)
    for p in range(128):
        m0 = (p // 16) * 16
        sel[p, m0:m0 + 16] = 1.0 / BS
    return sel


def kernel(**inputs):
    in_caps = np.ascontiguousarray(inputs["in_caps"], dtype=np.float32)
    W = np.ascontiguousarray(inputs["W"], dtype=np.float32)
    assert in_caps.shape == (BS, R, I) and W.shape == (R, J, O, I)

    bf = np.float16
    Wt = np.ascontiguousarray(
        W.transpose(0, 3, 1, 2).reshape(R * I, J * O).astype(bf))
    uT = np.ascontiguousarray(
        in_caps.transpose(1, 2, 0).reshape(R * I, BS).astype(bf))
    un = np.ascontiguousarray(in_caps.reshape(BS, R * I).astype(bf))
    sel = _make_sel().astype(np.float16)

    in_maps = []
    for k in range(N_CORES):
        rows = slice(k * K_LOC, (k + 1) * K_LOC)
        # p-major: [chunk, partition, free] -> [partition, chunk*free]
        wt_pm = Wt[rows].reshape(NCHUNK, 128, JO).transpose(1, 0, 2)
        ut_pm = uT[rows].reshape(NCHUNK, 128, BS).transpose(1, 0, 2)
        in_maps.append({
            "wt": np.ascontiguousarray(wt_pm.reshape(128, NCHUNK * JO)),
            "ut": np.ascontiguousarray(ut_pm.reshape(128, NCHUNK * BS)),
            "un": np.ascontiguousarray(un[:, rows]),
            "sel": sel,
        })

    nc = _get_nc()
    res = run_bass_kernel_spmd(nc, in_maps, core_ids=list(range(N_CORES)),
                               trace=TRACE, trace_cores=TRACE_CORES)
    _NC_CACHE["last_result"] = res
    v = np.asarray(res.results[0]["vout"], dtype=np.float32)
    return v.reshape(BS, J, O, 1)


if __name__ == "__main__":
    rng = np.random.default_rng(0)
    ins = {
        "in_caps": rng.standard_normal((BS, R, I), dtype=np.float32),
        "W": rng.standard_normal((R, J, O, I), dtype=np.float32),
    }
    out = kernel(**ins)
    print(out.shape, out.dtype, np.abs(out).mean())


# revision 37
# speedup vs baseline: 1.0289x; 1.0289x over previous
"""DigitCaps dynamic-routing kernel for Trainium2, 8 NeuronCores (SPMD).

Problem:  in_caps [64, 2048, 16] f32, W [2048, 32, 32, 16] f32
          u_hat[b,r,j,o] = sum_i W[r,j,o,i] * in_caps[b,r,i]
          3 routing iterations:
            c = softmax_j(b_ij);  s[b,j,o] = sum_r c[r,j] u_hat[b,r,j,o]
            v = squash_o(s);      b_ij += (1/BS) sum_{b,o} u_hat v
          returns v[..., None]  -> [64, 32, 32, 1]

Strategy (per core, routes sharded 256/core; K = (r,i) = 4096 rows):
  * W resident in SBUF as fp16 Wt[(r,i),(j,o)], p-major, 8+8 slices on
    the two HWDGE queues; pass1-it0 streams behind the load.  u_hat is
    never materialized.
  * pass1: the K contraction is column-tiled - even chunks accumulate
    into PSUM partitions 0-63, odd chunks into 64-127, concurrently
    (~2x PE).  The shared banks mean no MM may use start=True, so the
    s tile is DVE-zeroed first and every MM accumulates via the
    per-element has_written bits.  Both halves go out in one [128,1024]
    fp16 AllReduce; they are summed after the readback (both halves
    land on partitions 0-63).  One AR per iteration; it0's uniform-c
    1/J is folded into the squash factor.
  * squash: processed per j-half straight off the readback DMAs (the
    h=0 G matmuls start on the first half); square on DVE in bf16
    (fp32 range + 2x mode; fp16 would overflow on it0's raw s).  A
    dense burst of junk matmuls keyed on the first s2 half runs during
    the squash so the PE HAM clock is high when the G phase starts.
  * pass2 windows are emitted stage-blocked in 2-batch superblocks
    (G matmuls | W.G mult | o-tree to 4 | 4 selector matmuls -> b_ij |
    softmax | wc), with pass1(it+1) chunk pairs interleaved so wc tiles
    are consumed close to creation.  wc = Wt (.) c runs on DVE in 2x
    packed-fp16 mode via a pair-duplicated c2 layout (the broadcast
    over o keeps a last AP dim of [stride 1, count 2]); alternate
    batches are split 2-chunks-gpsimd / 2-chunks-DVE.
  * ACT only ever runs Copy/Sqrt/Exp (2 table slots; dummy sqrt ops
    re-pin the Sqrt table off the squash critical path each iteration).
"""

import numpy as np
import ml_dtypes

import concourse.bacc as bacc
import concourse.mybir as mybir
import concourse.tile as tile
from concourse.bass_utils import run_bass_kernel_spmd

BS, R, J, I, O = 64, 2048, 32, 16, 32
NUM_IT = 3
N_CORES = 8
R_LOC = R // N_CORES            # 256 routes per core
K_LOC = R_LOC * I               # 4096 contraction rows per core
NCHUNK = K_LOC // 128           # 32 chunks (8 routes x 16 i each)
JO = J * O                      # 1024
F32 = mybir.dt.float32
BF16 = mybir.dt.bfloat16
FP16 = mybir.dt.float16
AX = mybir.AxisListType
ALU = mybir.AluOpType
ACTF = mybir.ActivationFunctionType

# per-batch engine for wc = W*c: 'd' DVE (c2 pair trick, 2x), 'g' gpsimd
WC_ENG = ["d", "m", "d", "m", "d", "m", "d", "m"]


def _build_nc():
    nc = bacc.Bacc(trn_type="TRN2", target_bir_lowering=False, debug=False,
                   num_devices=N_CORES)
    wt = nc.dram_tensor("wt", [128, NCHUNK * JO], FP16, kind="ExternalInput")
    ut = nc.dram_tensor("ut", [128, NCHUNK * BS], FP16, kind="ExternalInput")
    un = nc.dram_tensor("un", [BS, K_LOC], FP16, kind="ExternalInput")
    sel = nc.dram_tensor("sel", [128, 128], FP16, kind="ExternalInput")
    vout = nc.dram_tensor("vout", [BS, JO], F32, kind="ExternalOutput")
    cc_in = [nc.dram_tensor(f"cc_in{i}", [128, JO], FP16)
             for i in range(NUM_IT)]
    cc_out = [nc.dram_tensor(f"cc_out{i}", [128, JO], FP16,
                             addr_space="Shared") for i in range(NUM_IT)]
    rg = [list(range(N_CORES))]

    with tile.TileContext(nc) as tc:
        with (
            tc.tile_pool(name="big", bufs=1) as big,
            tc.tile_pool(name="wc", bufs=4) as wcp,
            tc.tile_pool(name="gsb", bufs=3) as gsbp,
            tc.tile_pool(name="tmp", bufs=2) as tmpp,
            tc.tile_pool(name="tb", bufs=4) as tbp,
            tc.tile_pool(name="small", bufs=1) as small,
            tc.tile_pool(name="sq2", bufs=2) as sq2p,
            tc.tile_pool(name="sps", bufs=1, space="PSUM") as spsp,
            tc.tile_pool(name="gps", bufs=2, space="PSUM") as gpsp,
            tc.tile_pool(name="bpsum", bufs=1, space="PSUM") as bpsum,
        ):
            # ---- resident tensors ----
            w_sb = big.tile([128, NCHUNK, JO], FP16)      # 64KB/part
            ut_sb = big.tile([128, NCHUNK, BS], FP16)
            un_sb = big.tile([BS, K_LOC], FP16)
            sel_sb = big.tile([128, 128], FP16)            # selector (1/64)
            e_rep = big.tile([128, NCHUNK, J], FP16)      # exp(b) scratch
            c_rep = big.tile([128, NCHUNK, J], FP16)      # c_ij replicated
            c2 = big.tile([128, NCHUNK * J, 2], FP16)     # c pair-duplicated
            b_acc = bpsum.tile([128, NCHUNK, J], F32)     # persistent b_ij
            dumm = big.tile([64, 2], F32)                  # act-table pin

            # ---- input loads (sync + scalar HWDGE queues) ----
            wt_v = wt.ap().rearrange("p (c f) -> p c f", f=JO)
            ut_v = ut.ap().rearrange("p (c f) -> p c f", f=BS)
            nc.sync.dma_start(out=ut_sb, in_=ut_v)
            nc.scalar.dma_start(out=sel_sb, in_=sel.ap())
            _dengs = [nc.sync, nc.scalar]
            for sl in range(16):
                _dengs[sl % 2].dma_start(
                    out=w_sb[:, 2 * sl:2 * sl + 2, :],
                    in_=wt_v[:, 2 * sl:2 * sl + 2, :])
            # un is only needed by pass2 (after AR0) -> load last.
            nc.scalar.dma_start(out=un_sb, in_=un.ap())
            # Pin ACT table slots (Sqrt / Exp; Copy lives in both sets).
            nc.vector.memset(dumm, 1.0)
            nc.scalar.sqrt(dumm[:, 0:1], dumm[:, 1:2])
            nc.scalar.activation(dumm[:, 0:1], dumm[:, 1:2], ACTF.Exp)

            def emit_p1_pair(k, s_ps, rhs_a=None, rhs_b=None):
                """Chunks (2k, 2k+1) -> s halves on PSUM partition halves
                (column-tiled, concurrent)."""
                ca, cb = 2 * k, 2 * k + 1
                first, last = (k == 0), (k == 15)
                if rhs_a is None:
                    rhs_a = w_sb[:, ca, :]
                    rhs_b = w_sb[:, cb, :]
                # The two column-tile halves interleave in the same PSUM
                # banks, so no MM may use start=True (it would clear the
                # other half's accumulation bits).  The tile is DVE-zeroed
                # before the first MM; every MM then accumulates (elements
                # with has_written unset are simply overwritten = added to
                # the memset zeros either way).
                for h in range(2):
                    hs = slice(h * 512, (h + 1) * 512)
                    nc.tensor.matmul(
                        out=s_ps[0:BS, hs], lhsT=ut_sb[:, ca, :],
                        rhs=rhs_a[:, hs], start=False, stop=False,
                        skip_group_check=True)
                    nc.tensor.matmul(
                        out=s_ps[BS:128, hs], lhsT=ut_sb[:, cb, :],
                        rhs=rhs_b[:, hs], start=False, stop=(last and h == 1),
                        skip_group_check=True)

            def emit_s_evac_ar(it, s_ps):
                """Evacuate both halves into one [128,1024] fp16 AR; the
                halves are summed after the AR readback."""
                s_a = small.tile([128, JO], FP16, tag="s_evac")
                nc.scalar.copy(s_a, s_ps)
                nc.scalar.dma_start(out=cc_in[it].ap(), in_=s_a)
                nc.gpsimd.collective_compute(
                    "AllReduce", ALU.add, replica_groups=rg,
                    ins=[cc_in[it].ap()], outs=[cc_out[it].ap()])
                # re-pin the Sqrt table off the squash critical path
                nc.scalar.sqrt(dumm[:, 0:1], dumm[:, 1:2])

            def emit_squash(it, final=False):
                """cc_out -> combine halves -> squash, processed per
                j-half so downstream h=0 work starts ~4us earlier; v fp16
                (f32 final).  s was AllReduced RAW; it0's uniform-c 1/J is
                folded into the squash factor."""
                s2 = sq2p.tile([BS, J, O], FP16, tag="s2")
                s2f = s2.rearrange("p j o -> p (j o)")
                if final:
                    vt = small.tile([BS, J, O], F32, tag="vf32")
                else:
                    vt = small.tile([BS, J, O], FP16, tag=f"vr{it % 2}")
                for q in range(2):
                    js = slice(q * 16, q * 16 + 16)
                    cs = slice(q * 512, (q + 1) * 512)
                    s2a = sq2p.tile([BS, 512], FP16, tag=f"s2a{q}")
                    s2b = sq2p.tile([BS, 512], FP16, tag=f"s2b{q}")
                    nc.sync.dma_start(out=s2a, in_=cc_out[it].ap()[0:BS, cs])
                    nc.scalar.dma_start(out=s2b,
                                        in_=cc_out[it].ap()[BS:128, cs])
                    nc.vector.tensor_tensor(out=s2f[:, cs], in0=s2a,
                                            in1=s2b, op=ALU.add)
                    if q == 0 and not final:
                        # HAM promoter: dense junk matmuls keyed on the
                        # first s2 half run during the rest of the squash
                        # so the PE clock is high when the G phase starts.
                        pro = gpsp.tile([128, JO], F32, tag="gps")
                        for w in range(12):
                            nc.tensor.matmul(
                                out=pro[0:BS, 0:512], lhsT=un_sb[:, 0:BS],
                                rhs=s2f[:, 0:512], start=True, stop=True)
                    # ss must not be fp16 (it0's raw s^2 overflows); bf16
                    # keeps fp32 range and the DVE 2x mode.
                    ss = small.tile([BS, 16, O], BF16, tag=f"ss{q}")
                    nc.vector.tensor_tensor(out=ss, in0=s2[:, js, :],
                                            in1=s2[:, js, :], op=ALU.mult)
                    sq = small.tile([BS, 16], F32, tag=f"sq{q}")
                    nc.vector.tensor_reduce(out=sq, in_=ss, axis=AX.X,
                                            op=ALU.add)
                    if it == 0:
                        sq1 = small.tile([BS, 16], F32, tag=f"sqs{q}")
                        nc.vector.tensor_scalar_mul(sq1, sq, 1.0 / (J * J))
                        sq = sq1
                    rt = small.tile([BS, 16], F32, tag=f"rt{q}")
                    nc.scalar.sqrt(rt, sq)
                    op1 = small.tile([BS, 16], F32, tag=f"op{q}")
                    nc.vector.tensor_scalar_add(op1, sq, 1.0)
                    rden = small.tile([BS, 16], F32, tag=f"rd{q}")
                    nc.vector.reciprocal(rden, op1)
                    fac = small.tile([BS, 16], F32, tag=f"fa{q}")
                    nc.vector.tensor_tensor(out=fac, in0=rt, in1=rden,
                                            op=ALU.mult)
                    if it == 0:
                        fac1 = small.tile([BS, 16], F32, tag=f"fb{q}")
                        nc.vector.tensor_scalar_mul(fac1, fac, 1.0 / J)
                        fac = fac1
                    fac_b = fac.unsqueeze(2).broadcast_to([BS, 16, O])
                    nc.vector.tensor_tensor(out=vt[:, js, :],
                                            in0=s2[:, js, :], in1=fac_b,
                                            op=ALU.mult)
                    if final:
                        deng = nc.sync if q == 0 else nc.scalar
                        deng.dma_start(
                            out=vout.ap()[:, cs],
                            in_=vt.rearrange("p j o -> p (j o)")[:, cs])
                return vt.rearrange("p j o -> p (j o)")

            def emit_g_batch(b, v_r, g_tiles):
                """G = un.T @ v for the 4 chunks of batch b (PE) + ACT
                evacuation to fp16 SBUF."""
                c0 = 4 * b
                g_sb = gsbp.tile([128, 4, JO], FP16)
                for ci in range(4):
                    g_ps = gpsp.tile([128, JO], F32, tag="gps")
                    for h in range(2):
                        for m in range(2):
                            nc.tensor.matmul(
                                out=g_ps[m * 64:(m + 1) * 64,
                                         h * 512:(h + 1) * 512],
                                lhsT=un_sb[:, (c0 + ci) * 128 + m * 64:
                                           (c0 + ci) * 128 + m * 64 + 64],
                                rhs=v_r[:, h * 512:(h + 1) * 512],
                                start=True, stop=True)
                    nc.scalar.copy(g_sb[:, ci, :], g_ps)
                g_tiles[b] = g_sb

            def emit_mult_tree(b, g_tiles, tb_tiles):
                """tmp = W (.) G, fp16 tree to o=4 (DVE)."""
                c0 = 4 * b
                tmp = tmpp.tile([128, 4, JO], FP16)
                nc.vector.tensor_tensor(out=tmp, in0=w_sb[:, c0:c0 + 4, :],
                                        in1=g_tiles[b], op=ALU.mult)
                t0 = tmp.rearrange("p c (j o) -> p (c j) o", o=O)
                ta = tmpp.tile([128, 128, 16], FP16, tag="ta")
                nc.vector.tensor_tensor(out=ta, in0=t0[:, :, 0:16],
                                        in1=t0[:, :, 16:32], op=ALU.add)
                tb = tmpp.tile([128, 128, 8], FP16, tag="tb")
                nc.vector.tensor_tensor(out=tb, in0=ta[:, :, 0:8],
                                        in1=ta[:, :, 8:16], op=ALU.add)
                tcq = tbp.tile([128, 128, 4], FP16)
                nc.vector.tensor_tensor(out=tcq, in0=tb[:, :, 0:4],
                                        in1=tb[:, :, 4:8], op=ALU.add)
                tb_tiles[b] = tcq

            def emit_sel(it, b, tb_tiles):
                c0 = 4 * b
                tcq = tb_tiles[b]
                for oo in range(4):
                    nc.tensor.matmul(
                        out=b_acc[:, c0:c0 + 4, :], lhsT=sel_sb,
                        rhs=tcq[:, :, oo],
                        start=(it == 0 and c0 % 16 == 0 and oo == 0),
                        stop=(it == NUM_IT - 2 and c0 % 16 == 12
                              and oo == 3),
                        skip_group_check=True)

            def emit_softmax(b):
                c0 = 4 * b
                nc.scalar.activation(e_rep[:, c0:c0 + 4, :],
                                     b_acc[:, c0:c0 + 4, :], ACTF.Exp)
                esum = tmpp.tile([128, 4], F32, tag="esum")
                nc.vector.tensor_reduce(
                    out=esum, in_=e_rep[:, c0:c0 + 4, :],
                    axis=AX.X, op=ALU.add)
                erec = tmpp.tile([128, 4], F32, tag="erec")
                nc.vector.reciprocal(erec, esum)
                for cc in range(4):
                    nc.scalar.mul(c_rep[:, c0 + cc, :],
                                  e_rep[:, c0 + cc, :],
                                  erec[:, cc:cc + 1])

            def emit_c2_wc(b, wc_tiles):
                c0 = 4 * b
                wc_t = wcp.tile([128, 4, JO], FP16)
                nd = 4 if WC_ENG[b] == "d" else 2   # chunks on DVE (rest gp)
                if nd < 4:
                    # leading chunks on gpsimd, broadcast-direct
                    nc.gpsimd.tensor_tensor(
                        out=wc_t[:, 0:4 - nd, :].rearrange(
                            "p c (j o) -> p c j o", o=O),
                        in0=w_sb[:, c0:c0 + 4 - nd, :].rearrange(
                            "p c (j o) -> p c j o", o=O),
                        in1=c_rep[:, c0:c0 + 4 - nd, :].unsqueeze(3)
                            .broadcast_to([128, 4 - nd, J, O]),
                        op=ALU.mult)
                cd = c0 + 4 - nd                     # first DVE chunk
                cs = cd * J
                nc.scalar.copy(
                    c2[:, cs:cs + nd * J, :],
                    c_rep[:, cd:cd + nd, :]
                        .rearrange("p c j -> p (c j)").unsqueeze(2)
                        .broadcast_to([128, nd * J, 2]))
                nc.vector.tensor_tensor(
                    out=wc_t[:, 4 - nd:4, :].rearrange(
                        "p c (j oh ol) -> p (c j) oh ol", oh=16, ol=2),
                    in0=w_sb[:, cd:cd + nd, :].rearrange(
                        "p c (j oh ol) -> p (c j) oh ol", oh=16, ol=2),
                    in1=c2[:, cs:cs + nd * J, :].unsqueeze(2)
                        .broadcast_to([128, nd * J, 16, 2]),
                    op=ALU.mult)
                wc_tiles[b] = wc_t

            def emit_p1_batch(b, s_ps, wc_tiles):
                wc_t = wc_tiles[b]
                emit_p1_pair(2 * b, s_ps,
                             rhs_a=wc_t[:, 0, :], rhs_b=wc_t[:, 1, :])
                emit_p1_pair(2 * b + 1, s_ps,
                             rhs_a=wc_t[:, 2, :], rhs_b=wc_t[:, 3, :])

            def emit_window(it, v_r):
                """pass2(it) fused with pass1(it+1), emitted stage-blocked
                in 2-batch superblocks; returns new s psum."""
                s_ps = spsp.tile([128, JO], F32, tag="sps")
                nc.vector.memset(s_ps, 0.0)
                g_tiles, tb_tiles, wc_tiles = {}, {}, {}

                def stage(bs_g, bs_post, bs_p1):
                    for b in bs_g:
                        emit_g_batch(b, v_r, g_tiles)
                    for b in bs_post:
                        emit_mult_tree(b, g_tiles, tb_tiles)
                    for b in bs_post:
                        emit_sel(it, b, tb_tiles)
                    for b in bs_post:
                        emit_softmax(b)
                    for b in bs_post:
                        emit_c2_wc(b, wc_tiles)
                    for b in bs_p1:
                        emit_p1_batch(b, s_ps, wc_tiles)

                sbs = [(0, 1), (2, 3), (4, 5), (6, 7)]
                for s in range(6):
                    stage(sbs[s] if s < 4 else (),
                          sbs[s - 1] if 1 <= s < 5 else (),
                          sbs[s - 2] if s >= 2 else ())
                return s_ps

            # ---- iteration 0: pass1 on raw W, streaming behind the load
            s_ps = spsp.tile([128, JO], F32, tag="sps")
            nc.vector.memset(s_ps, 0.0)
            for k in range(16):
                emit_p1_pair(k, s_ps)
            emit_s_evac_ar(0, s_ps)

            for it in range(NUM_IT - 1):
                v_r = emit_squash(it)
                s_ps = emit_window(it, v_r)
                emit_s_evac_ar(it + 1, s_ps)

            # final squash (writes vout per half internally).
            emit_squash(NUM_IT - 1, final=True)
    nc.finalize()
    return nc


_NC_CACHE = {}
TRACE = False
TRACE_CORES = None


def _get_nc():
    if "nc" not in _NC_CACHE:
        _NC_CACHE["nc"] = _build_nc()
    return _NC_CACHE["nc"]


def _make_sel():
    sel = np.zeros((128, 128), np.float32)
    for p in range(128):
        m0 = (p // 16) * 16
        sel[p, m0:m0 + 16] = 1.0 / BS
    return sel


def kernel(**inputs):
    in_caps = np.ascontiguousarray(inputs["in_caps"], dtype=np.float32)
    W = np.ascontiguousarray(inputs["W"], dtype=np.float32)
    assert in_caps.shape == (BS, R, I) and W.shape == (R, J, O, I)

    bf = np.float16
    Wt = np.ascontiguousarray(
        W.transpose(0, 3, 1, 2).reshape(R * I, J * O).astype(bf))
    uT = np.ascontiguousarray(
        in_caps.transpose(1, 2, 0).reshape(R * I, BS).astype(bf))
    un = np.ascontiguousarray(in_caps.reshape(BS, R * I).astype(bf))
    sel = _make_sel().astype(np.float16)

    in_maps = []
    for k in range(N_CORES):
        rows = slice(k * K_LOC, (k + 1) * K_LOC)
        # p-major: [chunk, partition, free] -> [partition, chunk*free]
        wt_pm = Wt[rows].reshape(NCHUNK, 128, JO).transpose(1, 0, 2)
        ut_pm = uT[rows].reshape(NCHUNK, 128, BS).transpose(1, 0, 2)
        in_maps.append({
            "wt": np.ascontiguousarray(wt_pm.reshape(128, NCHUNK * JO)),
            "ut": np.ascontiguousarray(ut_pm.reshape(128, NCHUNK * BS)),
            "un": np.ascontiguousarray(un[:, rows]),
            "sel": sel,
        })

    nc = _get_nc()
    res = run_bass_kernel_spmd(nc, in_maps, core_ids=list(range(N_CORES)),
                               trace=TRACE, trace_cores=TRACE_CORES)
    _NC_CACHE["last_result"] = res
    v = np.asarray(res.results[0]["vout"], dtype=np.float32)
    return v.reshape(BS, J, O, 1)


if __name__ == "__main__":
    rng = np.random.default_rng(0)
    ins = {
        "in_caps": rng.standard_normal((BS, R, I), dtype=np.float32),
        "W": rng.standard_normal((R, J, O, I), dtype=np.float32),
    }
    out = kernel(**ins)
    print(out.shape, out.dtype, np.abs(out).mean())
